# revision 1
# baseline (speedup 1.0000x reference)

# Trainium2 Bass kernel for MinConvExpLSTMCell.
#
# Math (linear-space reformulation of the reference's log-space scan):
#   y = conv3x3(x, W) + b; [f_gate, i_gate, h_tilde] = split(y)
#   diff = f_gate - i_gate = conv(x, W_f - W_i) + (b_f - b_i)
#   f = sigmoid(diff);  i = 1 - f
#   g = sigmoid(min(ht, 0)) + relu(ht)          (= g(h_tilde), exact identity)
#   h_t = f_t * h_{t-1} + i_t * g_t,  h_{-1} = g(h0)
#
# Sharding: 8 cores = 4 batches x 2 spatial halves (16 output rows each,
# 1 halo row). Conv = 9 accumulated matmuls per time step (K=64 in-ch,
# M=128 = [diff;ht] out-ch, N=512 px), bf16, row-tiled in pairs across PE
# row-groups (image duplicated on partitions 0-63 / 64-127).
# Recurrence: tensor_tensor_scan along a pixel-major/time-minor layout,
# segmented by 8 time steps, chained via a per-pixel init column.

import sys
import numpy as np

sys.path.insert(0, "/opt/trn_rl_repo")

import ml_dtypes
from contextlib import ExitStack

import concourse.bass as bass
import concourse.bacc as bacc
import concourse.mybir as mybir
from concourse.tile import TileContext
from concourse.bass_utils import run_bass_kernel_spmd

BF16 = ml_dtypes.bfloat16
B, T, C, H, W = 4, 64, 64, 32, 32
SEG = 8
NSEG = T // SEG
HP, WP = 18, 34            # padded shard rows/cols
RC = HP * WP               # 612
NPX = 16 * 32              # 512 output pixels per core
TS = SEG + 1               # 9 scan slots per pixel per segment
NF = NPX * TS              # 4608 scan free size
TAPS = [(r0, c0) for r0 in range(3) for c0 in range(3)]

_CACHE = {}


def _build():
    f32 = mybir.dt.float32
    bf16 = mybir.dt.bfloat16
    AF = mybir.ActivationFunctionType
    OP = mybir.AluOpType

    nc = bacc.Bacc()
    xs = nc.dram_tensor("xs", [T, C, RC], bf16, kind="ExternalInput")
    wt = nc.dram_tensor("wt", [128, 9 * 128], bf16, kind="ExternalInput")
    cst = nc.dram_tensor("cst", [64, 2 + NPX], f32, kind="ExternalInput")
    out = nc.dram_tensor("out", [NSEG, 64, NF], f32, kind="ExternalOutput")

    with TileContext(nc) as tc, ExitStack() as ctx:
        cpool = ctx.enter_context(tc.tile_pool(name="consts", bufs=1))
        xpool = ctx.enter_context(tc.tile_pool(name="x", bufs=2))
        pspool = ctx.enter_context(tc.tile_pool(name="ps", bufs=2, space="PSUM"))
        gpool = ctx.enter_context(tc.tile_pool(name="g", bufs=2))
        sigpool = ctx.enter_context(tc.tile_pool(name="sig", bufs=2))
        spool = ctx.enter_context(tc.tile_pool(name="s", bufs=1))
        rpool = ctx.enter_context(tc.tile_pool(name="r", bufs=1))
        ggpool = ctx.enter_context(tc.tile_pool(name="gg", bufs=1))
        ipool = ctx.enter_context(tc.tile_pool(name="i", bufs=1))
        upool = ctx.enter_context(tc.tile_pool(name="u", bufs=2))
        hpool = ctx.enter_context(tc.tile_pool(name="h", bufs=2))

        w_sb = cpool.tile([128, 9 * 128], bf16)
        nc.sync.dma_start(w_sb[:, :], wt[:, :])
        cst_sb = cpool.tile([64, 2 + NPX], f32)
        nc.sync.dma_start(cst_sb[:, :], cst[:, :])
        bd = cst_sb[:, 0:1]
        bh = cst_sb[:, 1:2]
        g0 = cst_sb[:, 2:2 + NPX]

        h_prev = None
        for s in range(NSEG):
            xt = xpool.tile([64, SEG * RC], bf16)
            src = xs[s * SEG:(s + 1) * SEG].rearrange("t c x -> c t x")
            nc.sync.dma_start(
                xt[0:64, :].rearrange("p (t x) -> p t x", t=SEG), src)
            xv = xt.rearrange("p (t r c) -> p t r c", t=SEG, r=HP, c=WP)

            G = gpool.tile([128, SEG * 512], bf16)
            for k in range(SEG):
                ps = pspool.tile([128, 512], f32)
                for j, (r0, c0) in enumerate(TAPS):
                    rhs = xv[0:64, k, r0:r0 + 16, c0:c0 + 32]
                    lhsT = w_sb[0:64, j * 128:(j + 1) * 128]
                    nc.tensor.matmul(
                        ps[:, :], lhsT, rhs,
                        start=(j == 0), stop=(j == 8))
                dst = G[:, k * 512:(k + 1) * 512]
                if k % 2 == 0:
                    nc.scalar.activation(dst, ps[:, :], AF.Copy)
                else:
                    nc.vector.tensor_copy(dst, ps[:, :])

            Gd = G[0:64, :]
            Gh = G[64:128, :]

            # r = relu(ht + bh)   (before min destroys Gh)
            R = rpool.tile([64, SEG * 512], bf16)
            nc.gpsimd.tensor_scalar(R[:, :], Gh, bh, 0.0, OP.add, OP.max)
            # Gh <- min(ht + bh, 0)  in place
            nc.vector.tensor_scalar(Gh, Gh, bh, 0.0, OP.add, OP.min)

            # f = sigmoid(diff + bd) -> fp32, scan layout (t-minor)
            SIG = sigpool.tile([64, NF], f32)
            sig9 = SIG.rearrange("p (px t) -> p t px", t=TS)
            nc.gpsimd.memset(sig9[:, 0, :], 0.0)  # scan-reset column
            nc.scalar.activation(sig9[:, 1:TS, :], Gd, AF.Sigmoid, bias=bd)

            # s = sigmoid(min(ht+bh,0)) -> bf16 dense (t-major)
            S = spool.tile([64, SEG * 512], bf16)
            nc.scalar.activation(S[:, :], Gh, AF.Sigmoid)

            # g = s + r -> scan layout
            GG = ggpool.tile([64, NF], bf16)
            gg9 = GG.rearrange("p (px t) -> p t px", t=TS)
            nc.gpsimd.tensor_tensor(gg9[:, 1:TS, :], S[:, :], R[:, :], OP.add)

            # i = 1 - f (dense over scan buffer; col0 junk unused)
            I = ipool.tile([64, NF], bf16)
            nc.vector.tensor_scalar(
                I[:, :], SIG[:, :], -1.0, 1.0, OP.mult, OP.add)

            # u = i * g -> fp32 scan layout (skip col0)
            U = upool.tile([64, NF], f32)
            u9 = U.rearrange("p (px t) -> p px t", t=TS)
            i9 = I.rearrange("p (px t) -> p px t", t=TS)
            gx9 = GG.rearrange("p (px t) -> p px t", t=TS)
            nc.gpsimd.tensor_tensor(
                u9[:, :, 1:TS], i9[:, :, 1:TS], gx9[:, :, 1:TS], OP.mult)

            # u col0 = h_{-1} for this segment (chains segments)
            if h_prev is None:
                nc.vector.tensor_copy(u9[:, :, 0], g0)
            else:
                hp9 = h_prev.rearrange("p (px t) -> p px t", t=TS)
                nc.vector.tensor_copy(u9[:, :, 0], hp9[:, :, SEG])

            # h = scan: state = (f * state) + u, per-pixel chains
            Ht = hpool.tile([64, NF], f32)
            nc.vector.tensor_tensor_scan(
                Ht[:, :], SIG[:, :], U[:, :], 0.0, OP.mult, OP.add)
            h_prev = Ht

            nc.sync.dma_start(out[s], Ht[:, :])
    nc.finalize()
    return nc


def _g0(h0):
    return np.where(h0 >= 0.0, h0 + 0.5, 1.0 / (1.0 + np.exp(-h0))).astype(np.float32)


def kernel(x, conv_w, conv_b, h0):
    x = np.asarray(x, np.float32)
    conv_w = np.asarray(conv_w, np.float32)
    conv_b = np.asarray(conv_b, np.float32)
    h0 = np.asarray(h0, np.float32)

    if "nc" not in _CACHE:
        _CACHE["nc"] = _build()
    nc = _CACHE["nc"]

    wd = conv_w[0:64] - conv_w[64:128]
    wh = conv_w[128:192]
    wcat = np.concatenate([wd, wh], 0)           # [128 out, 64 in, 3, 3]
    bd = conv_b[0:64] - conv_b[64:128]
    bh = conv_b[128:192]

    wt = np.zeros((128, 9 * 128), np.float32)
    for j, (r0, c0) in enumerate(TAPS):
        # lhsT[k, m] = wcat[m, k, r0, c0]
        wt[0:64, j * 128:(j + 1) * 128] = wcat[:, :, r0, c0].T
    wt = wt.astype(BF16)

    x4 = x.reshape(B, T, C, H, W)
    g0f = _g0(h0)                                 # [B, C, H, W]

    in_maps = []
    for c in range(8):
        b, half = c // 2, c % 2
        xsh = np.zeros((T, C, HP, WP), np.float32)
        if half == 0:
            xsh[:, :, 1:18, 1:33] = x4[b, :, :, 0:17, :]
        else:
            xsh[:, :, 0:17, 1:33] = x4[b, :, :, 15:32, :]
        xsh = xsh.reshape(T, C, RC).astype(BF16)
        g0c = g0f[b, :, 16 * half:16 * half + 16, :].reshape(64, NPX)
        cst = np.concatenate(
            [bd[:, None], bh[:, None], g0c], 1).astype(np.float32)
        in_maps.append({"xs": xsh, "wt": wt, "cst": cst})

    _CACHE["in_maps"] = in_maps
    res = run_bass_kernel_spmd(nc, in_maps, core_ids=list(range(8)))

    outf = np.empty((B, T, C, H, W), np.float32)
    for c in range(8):
        b, half = c // 2, c % 2
        arr = res.results[c]["out"]               # [NSEG, 64, NF]
        hseq = arr.reshape(NSEG, 64, NPX, TS)[:, :, :, 1:]
        hseq = hseq.transpose(0, 3, 1, 2).reshape(T, C, 16, 32)
        outf[b, :, :, 16 * half:16 * half + 16, :] = hseq
    return outf.reshape(B * T, C, H, W)



# revision 7
# speedup vs baseline: 1.7601x; 1.7601x over previous

# Trainium2 Bass kernel for MinConvExpLSTMCell.
#
# Math (linear-space reformulation of the reference's log-space scan):
#   y = conv3x3(x, W) + b; [f_gate, i_gate, h_tilde] = split(y)
#   diff = f_gate - i_gate = conv(x, W_f - W_i) + (b_f - b_i)
#   f = sigmoid(diff);  i = 1 - f = sigmoid(-diff)
#   g = max(sigmoid(ht), ht + 0.5)              (exact identity for g(ht))
#   h_t = f_t * h_{t-1} + i_t * g_t,  h_{-1} = g(h0)
#
# Sharding: 8 cores = 4 batches x 2 spatial halves (16 output rows each,
# 1 halo row). Conv: image duplicated on partitions 64-127 shifted one
# column, so K=128 matmuls cover two taps each -> 6 matmuls per step
# (3 paired + 3 with zero bottom weights), N=512 px, M=128=[diff;ht].
# Tap-major matmul order accumulates 4 time steps in 4 PSUM banks with a
# shared stationary weight load, keeping the PE continuously busy.
# Gates are computed straight out of PSUM by the scalar engine
# (sigmoid/identity with per-partition bias), vector does g/u products
# and the per-pixel tensor_tensor_scan (pixel-major, time-minor layout,
# f=0 reset column chains segments via a per-pixel init slot).

import sys
import numpy as np

sys.path.insert(0, "/opt/trn_rl_repo")

import ml_dtypes
from contextlib import ExitStack

import concourse.bass as bass
import concourse.bacc as bacc
import concourse.mybir as mybir
from concourse.tile import TileContext
from concourse.bass_utils import run_bass_kernel_spmd

BF16 = ml_dtypes.bfloat16
B, T, C, H, W = 4, 64, 64, 32, 32
SEG = 8
NSEG = T // SEG
HP, WP = 18, 35            # padded shard rows/cols
RC = HP * WP               # 630
NPX = 16 * 32              # 512 output pixels per core
TS = SEG + 1               # 9 scan slots per pixel per segment
NF = NPX * TS              # 4608 scan free size
# 6 matmuls: (window_row, window_col); col 0 pairs taps (dc=-1, dc=0)
# via the +1-col-shifted duplicate, col 2 covers dc=+1 (bottom zeroed).
WINS = [(r0, c0) for r0 in range(3) for c0 in (0, 2)]

_CACHE = {}


def _build():
    f32 = mybir.dt.float32
    bf16 = mybir.dt.bfloat16
    AF = mybir.ActivationFunctionType
    OP = mybir.AluOpType

    nc = bacc.Bacc()
    xs = nc.dram_tensor("xs", [128, T * RC], bf16, kind="ExternalInput")
    wt = nc.dram_tensor("wt", [128, 6 * 128], bf16, kind="ExternalInput")
    cst = nc.dram_tensor("cst", [128, 2 + NPX], f32, kind="ExternalInput")
    out = nc.dram_tensor("out", [NSEG, 64, NF], bf16, kind="ExternalOutput")

    with TileContext(nc) as tc, ExitStack() as ctx:
        cpool = ctx.enter_context(tc.tile_pool(name="consts", bufs=1))
        xpool = ctx.enter_context(tc.tile_pool(name="x", bufs=2))
        pspool = ctx.enter_context(tc.tile_pool(name="ps", bufs=2, space="PSUM"))
        sigpool = ctx.enter_context(tc.tile_pool(name="sig", bufs=2))
        ipool = ctx.enter_context(tc.tile_pool(name="i", bufs=2))
        gppool = ctx.enter_context(tc.tile_pool(name="gp", bufs=2))
        gpool = ctx.enter_context(tc.tile_pool(name="g", bufs=2))
        hpool = ctx.enter_context(tc.tile_pool(name="h", bufs=2))

        w_sb = cpool.tile([128, 6 * 128], bf16)
        nc.sync.dma_start(w_sb[:, :], wt[:, :])
        cst_sb = cpool.tile([128, 2 + NPX], f32)
        nc.sync.dma_start(cst_sb[:, :], cst[:, :])
        bias128 = cst_sb[:, 0:1]           # [bd; bh]
        nbd = cst_sb[0:64, 1:2]            # -bd
        bh05 = cst_sb[64:128, 1:2]         # bh + 0.5
        g0 = cst_sb[0:64, 2:2 + NPX]       # g(h0) per pixel

        h_prev = None
        for s in range(NSEG):
            xt = xpool.tile([128, SEG * RC], bf16)
            nc.sync.dma_start(xt[:, :], xs[:, s * SEG * RC:(s + 1) * SEG * RC])
            xv = xt.rearrange("p (t r c) -> p t r c", t=SEG, r=HP, c=WP)

            SIG = sigpool.tile([128, NF], bf16)   # top: f9, bottom: sigmoid(ht)9
            I9 = ipool.tile([64, NF], bf16)       # i9, then u9 in place (+col0=h_init)
            GP = gppool.tile([128, NF], bf16)     # bottom half: ht+bh+0.5
            G9 = gpool.tile([64, NF], bf16)       # g9

            sig_t = SIG.rearrange("p (px t) -> p t px", t=TS)
            i_t = I9.rearrange("p (px t) -> p t px", t=TS)
            g_t = GP.rearrange("p (px t) -> p t px", t=TS)

            # f reset column for the per-pixel scan chains (idle engine)
            nc.gpsimd.memset(sig_t[0:64, 0, :], 0.0)

            for half in range(2):
                ps = pspool.tile([128, 4 * 512], f32)
                for j, (r0, c0) in enumerate(WINS):
                    lhsT = w_sb[:, j * 128:(j + 1) * 128]
                    for k in range(4):
                        rhs = xv[:, half * 4 + k, r0:r0 + 16, c0:c0 + 32]
                        nc.tensor.matmul(
                            ps[:, k * 512:(k + 1) * 512], lhsT, rhs,
                            start=(j == 0), stop=(j == 5))
                psv = ps.rearrange("p (k x) -> p k x", k=4)
                lo, hi = 1 + 4 * half, 5 + 4 * half
                # [f9; sig(ht)9] = sigmoid(ps + [bd; bh])
                nc.scalar.activation(
                    sig_t[:, lo:hi, :], psv[:, :, :], AF.Sigmoid, bias=bias128)
                # i9 = sigmoid(-(diff + bd))
                nc.scalar.activation(
                    i_t[:, lo:hi, :], psv[0:64, :, :], AF.Sigmoid,
                    bias=nbd, scale=-1.0)
                # g pre: ht + bh + 0.5 (bottom half, same base partition as src)
                nc.scalar.activation(
                    g_t[64:128, lo:hi, :], psv[64:128, :, :], AF.Identity,
                    bias=bh05)

            # slot views, pixel-major with packed time for DVE 2x mode
            sig_px = SIG.rearrange("p (px t) -> p px t", t=TS)
            i_px = I9.rearrange("p (px t) -> p px t", t=TS)
            gp_px = GP.rearrange("p (px t) -> p px t", t=TS)
            g_px = G9.rearrange("p (px t) -> p px t", t=TS)

            # g = max(sigmoid(ht), ht + 0.5)
            nc.vector.tensor_tensor(
                g_px[:, :, 1:TS], sig_px[64:128, :, 1:TS],
                gp_px[64:128, :, 1:TS], OP.max)
            # u = i * g   (in place on I9)
            nc.vector.tensor_tensor(
                i_px[:, :, 1:TS], i_px[:, :, 1:TS], g_px[:, :, 1:TS], OP.mult)
            # u col0 = h_{-1} for this segment (chains segments)
            if h_prev is None:
                nc.vector.tensor_copy(i_px[:, :, 0], g0)
            else:
                hp_px = h_prev.rearrange("p (px t) -> p px t", t=TS)
                nc.vector.tensor_copy(i_px[:, :, 0], hp_px[:, :, SEG])

            # h = scan: state = (f * state) + u, per-pixel chains
            Ht = hpool.tile([64, NF], bf16)
            nc.vector.tensor_tensor_scan(
                Ht[:, :], SIG[0:64, :], I9[:, :], 0.0, OP.mult, OP.add)
            h_prev = Ht

            nc.sync.dma_start(out[s], Ht[:, :])
    nc.finalize()
    return nc


def _g0(h0):
    return np.where(h0 >= 0.0, h0 + 0.5, 1.0 / (1.0 + np.exp(-h0))).astype(np.float32)


def kernel(x, conv_w, conv_b, h0):
    x = np.asarray(x, np.float32)
    conv_w = np.asarray(conv_w, np.float32)
    conv_b = np.asarray(conv_b, np.float32)
    h0 = np.asarray(h0, np.float32)

    if "nc" not in _CACHE:
        _CACHE["nc"] = _build()
    nc = _CACHE["nc"]

    wd = conv_w[0:64] - conv_w[64:128]
    wh = conv_w[128:192]
    wcat = np.concatenate([wd, wh], 0)           # [128 out, 64 in, 3, 3]
    bd = conv_b[0:64] - conv_b[64:128]
    bh = conv_b[128:192]

    # lhsT per window: [K=128, M=128]; K rows 0-63 = base image (tap dc=c0-1),
    # rows 64-127 = +1-col-shifted dup (tap dc=c0); c0=2 bottom zeroed.
    wt = np.zeros((128, 6 * 128), np.float32)
    for j, (r0, c0) in enumerate(WINS):
        blk = wt[:, j * 128:(j + 1) * 128]
        # base half reads padded col w=c0.. -> image col offset c0-1 -> tap col index c0
        blk[0:64, :] = wcat[:, :, r0, c0].T
        if c0 == 0:
            # dup half shifted +1 col -> tap col offset 0 -> tap col index 1
            blk[64:128, :] = wcat[:, :, r0, 1].T
    wt = wt.astype(BF16)

    x4 = x.reshape(B, T, C, H, W)
    g0f = _g0(h0)                                 # [B, C, H, W]

    in_maps = []
    for c in range(8):
        b, half = c // 2, c % 2
        xsh = np.zeros((128, T, HP, WP), np.float32)
        if half == 0:
            xsh[0:64, :, 1:18, 1:33] = x4[b].transpose(1, 0, 2, 3)[:, :, 0:17, :]
        else:
            xsh[0:64, :, 0:17, 1:33] = x4[b].transpose(1, 0, 2, 3)[:, :, 15:32, :]
        # duplicate shifted one column left: dup[.., w] = base[.., w+1]
        xsh[64:128, :, :, 0:WP - 1] = xsh[0:64, :, :, 1:WP]
        xsh = xsh.reshape(128, T * RC).astype(BF16)
        g0c = g0f[b, :, 16 * half:16 * half + 16, :].reshape(64, NPX)
        cstc = np.zeros((128, 2 + NPX), np.float32)
        cstc[0:64, 0] = bd
        cstc[64:128, 0] = bh
        cstc[0:64, 1] = -bd
        cstc[64:128, 1] = bh + 0.5
        cstc[0:64, 2:] = g0c
        in_maps.append({"xs": xsh, "wt": wt, "cst": cstc})

    _CACHE["in_maps"] = in_maps
    res = run_bass_kernel_spmd(nc, in_maps, core_ids=list(range(8)))

    outf = np.empty((B, T, C, 32, 32), np.float32)
    for c in range(8):
        b, half = c // 2, c % 2
        arr = np.asarray(res.results[c]["out"], dtype=np.float32)  # [NSEG, 64, NF]
        hseq = arr.reshape(NSEG, 64, NPX, TS)[:, :, :, 1:]
        hseq = hseq.transpose(0, 3, 1, 2).reshape(T, C, 16, 32)
        outf[b, :, :, 16 * half:16 * half + 16, :] = hseq
    return outf.reshape(B * T, C, 32, 32)


# revision 9
# speedup vs baseline: 3.4571x; 1.9641x over previous

# Trainium2 Bass kernel for MinConvExpLSTMCell.
#
# Math (linear-space reformulation of the reference's log-space scan):
#   y = conv3x3(x, W) + b; [f_gate, i_gate, h_tilde] = split(y)
#   diff = f_gate - i_gate = conv(x, W_f - W_i) + (b_f - b_i)
#   f = sigmoid(diff);  i = 1 - f = sigmoid(-diff)
#   g = max(sigmoid(ht), ht + 0.5)              (exact identity for g(ht))
#   h_t = f_t * h_{t-1} + i_t * g_t,  h_{-1} = g(h0)
#
# Sharding: 8 cores = 4 batches x 2 spatial halves (16 output rows each,
# 1 halo row). Conv: image duplicated on partitions 64-127 shifted one
# column, so K=128 matmuls cover two taps each -> 6 matmuls per step
# (3 paired + 3 with zero bottom weights), N=512 px, M=128=[diff;ht].
# Tap-major matmul order accumulates 4 time steps in 4 PSUM banks.
# PSUM is drained once by the vector engine (bias fused); sigmoids run
# on the scalar engine from SBUF. All gate tensors are pixel-SPLIT:
# pixels 0-255 on partitions 0-63, pixels 256-511 on partitions 64-127
# (per channel), so the per-pixel tensor_tensor_scan runs on all 128
# DVE lanes ([128, 2304] instead of [64, 4608]). Scan layout is
# pixel-major, time-minor with an f=0 reset column chaining segments
# via a per-pixel init slot.

import sys
import numpy as np

sys.path.insert(0, "/opt/trn_rl_repo")

import ml_dtypes
from contextlib import ExitStack

import concourse.bass as bass
import concourse.bacc as bacc
import concourse.mybir as mybir
from concourse.tile import TileContext
from concourse.bass_utils import run_bass_kernel_spmd

BF16 = ml_dtypes.bfloat16
B, T, C, H, W = 4, 64, 64, 32, 32
SEG = 8
NSEG = T // SEG
HP, WP = 18, 35            # padded shard rows/cols
RC = HP * WP               # 630
NPX = 16 * 32              # 512 output pixels per core
HPX = NPX // 2             # 256 pixels per partition-half
TS = SEG + 1               # 9 scan slots per pixel per segment
NF = NPX * TS              # 4608 dense gate free size (on 64 partitions)
NF2 = HPX * TS             # 2304 pixel-split free size (on 128 partitions)
# 6 matmuls: (window_row, window_col); col 0 pairs taps (dc=-1, dc=0)
# via the +1-col-shifted duplicate, col 2 covers dc=+1 (bottom zeroed).
WINS = [(r0, c0) for r0 in range(3) for c0 in (0, 2)]

_CACHE = {}


def _build():
    f32 = mybir.dt.float32
    bf16 = mybir.dt.bfloat16
    AF = mybir.ActivationFunctionType
    OP = mybir.AluOpType

    nc = bacc.Bacc()
    xs = nc.dram_tensor("xs", [128, T * RC], bf16, kind="ExternalInput")
    wt = nc.dram_tensor("wt", [128, 6 * 128], bf16, kind="ExternalInput")
    cst = nc.dram_tensor("cst", [128, 2 + HPX], f32, kind="ExternalInput")
    out = nc.dram_tensor("out", [NSEG, 128, NF2], bf16, kind="ExternalOutput")

    with TileContext(nc) as tc, ExitStack() as ctx:
        cpool = ctx.enter_context(tc.tile_pool(name="consts", bufs=1))
        xpool = ctx.enter_context(tc.tile_pool(name="x", bufs=2))
        pspool = ctx.enter_context(tc.tile_pool(name="ps", bufs=2, space="PSUM"))
        ypool = ctx.enter_context(tc.tile_pool(name="y", bufs=2))
        spool = ctx.enter_context(tc.tile_pool(name="s", bufs=2))
        fpool = ctx.enter_context(tc.tile_pool(name="f", bufs=2))
        ipool = ctx.enter_context(tc.tile_pool(name="i", bufs=2))
        gpool = ctx.enter_context(tc.tile_pool(name="g", bufs=2))
        hpool = ctx.enter_context(tc.tile_pool(name="h", bufs=2))

        w_sb = cpool.tile([128, 6 * 128], bf16)
        nc.sync.dma_start(w_sb[:, :], wt[:, :])
        cst_sb = cpool.tile([128, 2 + HPX], f32)
        nc.sync.dma_start(cst_sb[:, :], cst[:, :])
        biasp = cst_sb[:, 0:1]             # [bd; bh + 0.5]
        mhalf = cst_sb[64:128, 1:2]        # -0.5
        g0 = cst_sb[:, 2:2 + HPX]          # g(h0), pixel-split

        h_prev = None
        for s in range(NSEG):
            xt = xpool.tile([128, SEG * RC], bf16)
            nc.sync.dma_start(xt[:, :], xs[:, s * SEG * RC:(s + 1) * SEG * RC])
            xv = xt.rearrange("p (t r c) -> p t r c", t=SEG, r=HP, c=WP)

            Y9 = ypool.tile([128, NF], bf16)    # top: diff+bd, bottom: ht+bh+0.5
            y_t = Y9.rearrange("p (px t) -> p t px", t=TS)

            for half in range(2):
                ps = pspool.tile([128, 4 * 512], f32)
                for j, (r0, c0) in enumerate(WINS):
                    lhsT = w_sb[:, j * 128:(j + 1) * 128]
                    for k in range(4):
                        rhs = xv[:, half * 4 + k, r0:r0 + 16, c0:c0 + 32]
                        nc.tensor.matmul(
                            ps[:, k * 512:(k + 1) * 512], lhsT, rhs,
                            start=(j == 0), stop=(j == 5))
                psv = ps.rearrange("p (k x) -> p k x", k=4)
                lo, hi = 1 + 4 * half, 5 + 4 * half
                # single PSUM drain, bias fused (vector)
                nc.vector.tensor_scalar(
                    y_t[:, lo:hi, :], psv[:, :, :], biasp, None, OP.add)

            # pixel-split gate tiles: px 0-255 on partitions 0-63,
            # px 256-511 on partitions 64-127
            F2 = fpool.tile([128, NF2], bf16)
            I2 = ipool.tile([128, NF2], bf16)
            G2 = gpool.tile([128, NF2], bf16)
            S9 = spool.tile([128, NF], bf16)    # bottom half only

            y_px = Y9.rearrange("p (px t) -> p px t", t=TS)
            s_px = S9.rearrange("p (px t) -> p px t", t=TS)
            f_px = F2.rearrange("p (px t) -> p px t", t=TS)
            i_px = I2.rearrange("p (px t) -> p px t", t=TS)
            g_px = G2.rearrange("p (px t) -> p px t", t=TS)

            # f reset column for the per-pixel scan chains (idle engine)
            nc.gpsimd.memset(f_px[:, :, 0], 0.0)

            # f = sigmoid(diff + bd), split into pixel halves (scalar)
            nc.scalar.activation(
                f_px[0:64, :, 1:TS], y_px[0:64, 0:HPX, 1:TS], AF.Sigmoid)
            nc.scalar.activation(
                f_px[64:128, :, 1:TS], y_px[0:64, HPX:NPX, 1:TS], AF.Sigmoid)
            # i = sigmoid(-(diff + bd))
            nc.scalar.activation(
                i_px[0:64, :, 1:TS], y_px[0:64, 0:HPX, 1:TS], AF.Sigmoid,
                scale=-1.0)
            nc.scalar.activation(
                i_px[64:128, :, 1:TS], y_px[0:64, HPX:NPX, 1:TS], AF.Sigmoid,
                scale=-1.0)
            # s = sigmoid(ht + bh) = sigmoid((ht + bh + 0.5) - 0.5)
            nc.scalar.activation(
                s_px[64:128, :, 1:TS], y_px[64:128, :, 1:TS], AF.Sigmoid,
                bias=mhalf)

            # g = max(sigmoid(ht), ht + bh + 0.5), pixel-split (vector)
            nc.vector.tensor_tensor(
                g_px[0:64, :, 1:TS], s_px[64:128, 0:HPX, 1:TS],
                y_px[64:128, 0:HPX, 1:TS], OP.max)
            nc.vector.tensor_tensor(
                g_px[64:128, :, 1:TS], s_px[64:128, HPX:NPX, 1:TS],
                y_px[64:128, HPX:NPX, 1:TS], OP.max)
            # u = i * g   (in place on I2)
            nc.vector.tensor_tensor(
                i_px[:, :, 1:TS], i_px[:, :, 1:TS], g_px[:, :, 1:TS], OP.mult)

            # u col0 = h_{-1} for this segment (chains segments; scalar)
            if h_prev is None:
                nc.scalar.activation(i_px[:, :, 0], g0, AF.Copy)
            else:
                hp_px = h_prev.rearrange("p (px t) -> p px t", t=TS)
                nc.scalar.activation(i_px[:, :, 0], hp_px[:, :, SEG], AF.Copy)

            # h = scan: state = (f * state) + u, per-pixel chains, 128 lanes
            H2 = hpool.tile([128, NF2], bf16)
            nc.vector.tensor_tensor_scan(
                H2[:, :], F2[:, :], I2[:, :], 0.0, OP.mult, OP.add)
            h_prev = H2

            nc.sync.dma_start(out[s], H2[:, :])
    nc.finalize()
    return nc


def _g0(h0):
    return np.where(h0 >= 0.0, h0 + 0.5, 1.0 / (1.0 + np.exp(-h0))).astype(np.float32)


def kernel(x, conv_w, conv_b, h0):
    x = np.asarray(x, np.float32)
    conv_w = np.asarray(conv_w, np.float32)
    conv_b = np.asarray(conv_b, np.float32)
    h0 = np.asarray(h0, np.float32)

    if "nc" not in _CACHE:
        _CACHE["nc"] = _build()
    nc = _CACHE["nc"]

    wd = conv_w[0:64] - conv_w[64:128]
    wh = conv_w[128:192]
    wcat = np.concatenate([wd, wh], 0)           # [128 out, 64 in, 3, 3]
    bd = conv_b[0:64] - conv_b[64:128]
    bh = conv_b[128:192]

    # lhsT per window: [K=128, M=128]; K rows 0-63 = base image (tap dc=c0-1),
    # rows 64-127 = +1-col-shifted dup (tap dc=c0); c0=2 bottom zeroed.
    wt = np.zeros((128, 6 * 128), np.float32)
    for j, (r0, c0) in enumerate(WINS):
        blk = wt[:, j * 128:(j + 1) * 128]
        blk[0:64, :] = wcat[:, :, r0, c0].T
        if c0 == 0:
            blk[64:128, :] = wcat[:, :, r0, 1].T
    wt = wt.astype(BF16)

    x4 = x.reshape(B, T, C, H, W)
    g0f = _g0(h0)                                 # [B, C, H, W]

    in_maps = []
    for c in range(8):
        b, half = c // 2, c % 2
        xsh = np.zeros((128, T, HP, WP), np.float32)
        if half == 0:
            xsh[0:64, :, 1:18, 1:33] = x4[b].transpose(1, 0, 2, 3)[:, :, 0:17, :]
        else:
            xsh[0:64, :, 0:17, 1:33] = x4[b].transpose(1, 0, 2, 3)[:, :, 15:32, :]
        # duplicate shifted one column left: dup[.., w] = base[.., w+1]
        xsh[64:128, :, :, 0:WP - 1] = xsh[0:64, :, :, 1:WP]
        xsh = xsh.reshape(128, T * RC).astype(BF16)
        # g(h0) pixel-split: rows p<64: ch p, px 0-255; rows 64+p: px 256-511
        g0c = g0f[b, :, 16 * half:16 * half + 16, :].reshape(64, NPX)
        cstc = np.zeros((128, 2 + HPX), np.float32)
        cstc[0:64, 0] = bd
        cstc[64:128, 0] = bh + 0.5
        cstc[:, 1] = -0.5
        cstc[0:64, 2:] = g0c[:, 0:HPX]
        cstc[64:128, 2:] = g0c[:, HPX:NPX]
        in_maps.append({"xs": xsh, "wt": wt, "cst": cstc})

    _CACHE["in_maps"] = in_maps
    res = run_bass_kernel_spmd(nc, in_maps, core_ids=list(range(8)))

    outf = np.empty((B, T, C, 32, 32), np.float32)
    for c in range(8):
        b, half = c // 2, c % 2
        arr = np.asarray(res.results[c]["out"], dtype=np.float32)  # [NSEG,128,NF2]
        hs = arr.reshape(NSEG, 2, 64, HPX, TS)[:, :, :, :, 1:]
        # -> [T, ch, grp, px]
        hs = hs.transpose(0, 4, 2, 1, 3).reshape(T, C, NPX)
        outf[b, :, :, 16 * half:16 * half + 16, :] = hs.reshape(T, C, 16, 32)
    return outf.reshape(B * T, C, 32, 32)


# revision 10
# speedup vs baseline: 5.3738x; 1.5544x over previous

# Trainium2 Bass kernel for MinConvExpLSTMCell.
#
# Math (linear-space reformulation of the reference's log-space scan):
#   y = conv3x3(x, W) + b; [f_gate, i_gate, h_tilde] = split(y)
#   diff = f_gate - i_gate = conv(x, W_f - W_i) + (b_f - b_i)
#   f = sigmoid(diff);  i = 1 - f = sigmoid(-diff)
#   g = max(sigmoid(ht), ht + 0.5)              (exact identity for g(ht))
#   h_t = f_t * h_{t-1} + i_t * g_t,  h_{-1} = g(h0)
#
# Sharding: 8 cores = 4 batches x 2 spatial halves (16 output rows each,
# 1 halo row). Conv: image duplicated on partitions 64-127 shifted one
# column, so K=128 matmuls cover two taps each -> 6 matmuls per step
# (3 paired + 3 with zero bottom weights), N=512 px, M=128=[diff;ht].
# Tap-major matmul order accumulates 4 time steps in 4 PSUM banks.
# PSUM is drained once by the vector engine (bias fused); sigmoids run
# on the scalar engine from SBUF. All gate tensors are pixel-SPLIT:
# pixels 0-255 on partitions 0-63, pixels 256-511 on partitions 64-127
# (per channel), so the per-pixel tensor_tensor_scan runs on all 128
# DVE lanes ([128, 2304] instead of [64, 4608]). Scan layout is
# pixel-major, time-minor with an f=0 reset column chaining segments
# via a per-pixel init slot.

import sys
import numpy as np

sys.path.insert(0, "/opt/trn_rl_repo")

import ml_dtypes
from contextlib import ExitStack

import concourse.bass as bass
import concourse.bacc as bacc
import concourse.mybir as mybir
from concourse.tile import TileContext
from concourse.bass_utils import run_bass_kernel_spmd

BF16 = ml_dtypes.bfloat16
B, T, C, H, W = 4, 64, 64, 32, 32
SEG = 8
NSEG = T // SEG
HP, WP = 18, 35            # padded shard rows/cols
RC = HP * WP               # 630
NPX = 16 * 32              # 512 output pixels per core
HPX = NPX // 2             # 256 pixels per partition-half
TS = SEG + 1               # 9 scan slots per pixel per segment
NF = NPX * TS              # 4608 dense gate free size (on 64 partitions)
NF2 = HPX * TS             # 2304 pixel-split free size (on 128 partitions)
# 6 matmuls: (window_row, window_col); col 0 pairs taps (dc=-1, dc=0)
# via the +1-col-shifted duplicate, col 2 covers dc=+1 (bottom zeroed).
WINS = [(r0, c0) for r0 in range(3) for c0 in (0, 2)]

_CACHE = {}


def _build():
    f32 = mybir.dt.float32
    bf16 = mybir.dt.bfloat16
    AF = mybir.ActivationFunctionType
    OP = mybir.AluOpType

    nc = bacc.Bacc()
    xs = nc.dram_tensor("xs", [128, T * RC], bf16, kind="ExternalInput")
    wt = nc.dram_tensor("wt", [128, 6 * 128], bf16, kind="ExternalInput")
    cst = nc.dram_tensor("cst", [128, 2 + HPX], f32, kind="ExternalInput")
    out = nc.dram_tensor("out", [NSEG, 128, NF2], bf16, kind="ExternalOutput")

    with TileContext(nc) as tc, ExitStack() as ctx:
        cpool = ctx.enter_context(tc.tile_pool(name="consts", bufs=1))
        xpool = ctx.enter_context(tc.tile_pool(name="x", bufs=2))
        pspool = ctx.enter_context(tc.tile_pool(name="ps", bufs=2, space="PSUM"))
        ypool = ctx.enter_context(tc.tile_pool(name="y", bufs=2))
        spool = ctx.enter_context(tc.tile_pool(name="s", bufs=2))
        fpool = ctx.enter_context(tc.tile_pool(name="f", bufs=2))
        ipool = ctx.enter_context(tc.tile_pool(name="i", bufs=2))
        gpool = ctx.enter_context(tc.tile_pool(name="g", bufs=2))
        hpool = ctx.enter_context(tc.tile_pool(name="h", bufs=2))

        w_sb = cpool.tile([128, 6 * 128], bf16)
        nc.sync.dma_start(w_sb[:, :], wt[:, :])
        cst_sb = cpool.tile([128, 2 + HPX], f32)
        nc.sync.dma_start(cst_sb[:, :], cst[:, :])
        biasp = cst_sb[:, 0:1]             # [bd; bh + 0.5]
        mhalf = cst_sb[64:128, 1:2]        # -0.5
        g0 = cst_sb[:, 2:2 + HPX]          # g(h0), pixel-split

        h_prev = None
        for s in range(NSEG):
            xt = xpool.tile([128, SEG * RC], bf16)
            nc.sync.dma_start(xt[:, :], xs[:, s * SEG * RC:(s + 1) * SEG * RC])
            xv = xt.rearrange("p (t r c) -> p t r c", t=SEG, r=HP, c=WP)

            Y9 = ypool.tile([128, NF], bf16)    # top: diff+bd, bottom: ht+bh+0.5
            y_px2 = Y9.rearrange("p (px t) -> p px t", t=TS)

            for half in range(2):
                ps = pspool.tile([128, 4 * 512], f32)
                for j, (r0, c0) in enumerate(WINS):
                    lhsT = w_sb[:, j * 128:(j + 1) * 128]
                    for k in range(4):
                        rhs = xv[:, half * 4 + k, r0:r0 + 16, c0:c0 + 32]
                        nc.tensor.matmul(
                            ps[:, k * 512:(k + 1) * 512], lhsT, rhs,
                            start=(j == 0), stop=(j == 5))
                psx = ps.rearrange("p (k x) -> p x k", k=4)
                lo, hi = 1 + 4 * half, 5 + 4 * half
                # single PSUM drain, bias fused (vector); pixel-outer,
                # step-inner APs keep the innermost dim packed (fast path)
                nc.vector.tensor_scalar(
                    y_px2[:, :, lo:hi], psx[:, :, :], biasp, None, OP.add)

            # pixel-split gate tiles: px 0-255 on partitions 0-63,
            # px 256-511 on partitions 64-127
            F2 = fpool.tile([128, NF2], bf16)
            I2 = ipool.tile([128, NF2], bf16)
            G2 = gpool.tile([128, NF2], bf16)
            S9 = spool.tile([128, NF], bf16)    # bottom half only

            y_px = Y9.rearrange("p (px t) -> p px t", t=TS)
            s_px = S9.rearrange("p (px t) -> p px t", t=TS)
            f_px = F2.rearrange("p (px t) -> p px t", t=TS)
            i_px = I2.rearrange("p (px t) -> p px t", t=TS)
            g_px = G2.rearrange("p (px t) -> p px t", t=TS)

            # f reset column for the per-pixel scan chains (idle engine)
            nc.gpsimd.memset(f_px[:, :, 0], 0.0)

            # f = sigmoid(diff + bd), split into pixel halves (scalar)
            nc.scalar.activation(
                f_px[0:64, :, 1:TS], y_px[0:64, 0:HPX, 1:TS], AF.Sigmoid)
            nc.scalar.activation(
                f_px[64:128, :, 1:TS], y_px[0:64, HPX:NPX, 1:TS], AF.Sigmoid)
            # i = 1 - f (vector, single pixel-split op)
            nc.vector.tensor_scalar(
                i_px[:, :, 1:TS], f_px[:, :, 1:TS], -1.0, 1.0,
                OP.mult, OP.add)
            # s = sigmoid(ht + bh) = sigmoid((ht + bh + 0.5) - 0.5)
            nc.scalar.activation(
                s_px[64:128, :, 1:TS], y_px[64:128, :, 1:TS], AF.Sigmoid,
                bias=mhalf)

            # g = max(sigmoid(ht), ht + bh + 0.5), pixel-split (vector)
            nc.vector.tensor_tensor(
                g_px[0:64, :, 1:TS], s_px[64:128, 0:HPX, 1:TS],
                y_px[64:128, 0:HPX, 1:TS], OP.max)
            nc.vector.tensor_tensor(
                g_px[64:128, :, 1:TS], s_px[64:128, HPX:NPX, 1:TS],
                y_px[64:128, HPX:NPX, 1:TS], OP.max)
            # u = i * g   (in place on I2; gpsimd, otherwise idle)
            nc.gpsimd.tensor_tensor(
                i_px[:, :, 1:TS], i_px[:, :, 1:TS], g_px[:, :, 1:TS], OP.mult)

            # u col0 = h_{-1} for this segment (chains segments; scalar)
            if h_prev is None:
                nc.scalar.activation(i_px[:, :, 0], g0, AF.Copy)
            else:
                hp_px = h_prev.rearrange("p (px t) -> p px t", t=TS)
                nc.scalar.activation(i_px[:, :, 0], hp_px[:, :, SEG], AF.Copy)

            # h = scan: state = (f * state) + u, per-pixel chains, 128 lanes
            H2 = hpool.tile([128, NF2], bf16)
            nc.vector.tensor_tensor_scan(
                H2[:, :], F2[:, :], I2[:, :], 0.0, OP.mult, OP.add)
            h_prev = H2

            nc.sync.dma_start(out[s], H2[:, :])
    nc.finalize()
    return nc


def _g0(h0):
    return np.where(h0 >= 0.0, h0 + 0.5, 1.0 / (1.0 + np.exp(-h0))).astype(np.float32)


def kernel(x, conv_w, conv_b, h0):
    x = np.asarray(x, np.float32)
    conv_w = np.asarray(conv_w, np.float32)
    conv_b = np.asarray(conv_b, np.float32)
    h0 = np.asarray(h0, np.float32)

    if "nc" not in _CACHE:
        _CACHE["nc"] = _build()
    nc = _CACHE["nc"]

    wd = conv_w[0:64] - conv_w[64:128]
    wh = conv_w[128:192]
    wcat = np.concatenate([wd, wh], 0)           # [128 out, 64 in, 3, 3]
    bd = conv_b[0:64] - conv_b[64:128]
    bh = conv_b[128:192]

    # lhsT per window: [K=128, M=128]; K rows 0-63 = base image (tap dc=c0-1),
    # rows 64-127 = +1-col-shifted dup (tap dc=c0); c0=2 bottom zeroed.
    wt = np.zeros((128, 6 * 128), np.float32)
    for j, (r0, c0) in enumerate(WINS):
        blk = wt[:, j * 128:(j + 1) * 128]
        blk[0:64, :] = wcat[:, :, r0, c0].T
        if c0 == 0:
            blk[64:128, :] = wcat[:, :, r0, 1].T
    wt = wt.astype(BF16)

    x4 = x.reshape(B, T, C, H, W)
    g0f = _g0(h0)                                 # [B, C, H, W]

    in_maps = []
    for c in range(8):
        b, half = c // 2, c % 2
        xsh = np.zeros((128, T, HP, WP), np.float32)
        if half == 0:
            xsh[0:64, :, 1:18, 1:33] = x4[b].transpose(1, 0, 2, 3)[:, :, 0:17, :]
        else:
            xsh[0:64, :, 0:17, 1:33] = x4[b].transpose(1, 0, 2, 3)[:, :, 15:32, :]
        # duplicate shifted one column left: dup[.., w] = base[.., w+1]
        xsh[64:128, :, :, 0:WP - 1] = xsh[0:64, :, :, 1:WP]
        xsh = xsh.reshape(128, T * RC).astype(BF16)
        # g(h0) pixel-split: rows p<64: ch p, px 0-255; rows 64+p: px 256-511
        g0c = g0f[b, :, 16 * half:16 * half + 16, :].reshape(64, NPX)
        cstc = np.zeros((128, 2 + HPX), np.float32)
        cstc[0:64, 0] = bd
        cstc[64:128, 0] = bh + 0.5
        cstc[:, 1] = -0.5
        cstc[0:64, 2:] = g0c[:, 0:HPX]
        cstc[64:128, 2:] = g0c[:, HPX:NPX]
        in_maps.append({"xs": xsh, "wt": wt, "cst": cstc})

    _CACHE["in_maps"] = in_maps
    res = run_bass_kernel_spmd(nc, in_maps, core_ids=list(range(8)))

    outf = np.empty((B, T, C, 32, 32), np.float32)
    for c in range(8):
        b, half = c // 2, c % 2
        arr = np.asarray(res.results[c]["out"], dtype=np.float32)  # [NSEG,128,NF2]
        hs = arr.reshape(NSEG, 2, 64, HPX, TS)[:, :, :, :, 1:]
        # -> [T, ch, grp, px]
        hs = hs.transpose(0, 4, 2, 1, 3).reshape(T, C, NPX)
        outf[b, :, :, 16 * half:16 * half + 16, :] = hs.reshape(T, C, 16, 32)
    return outf.reshape(B * T, C, 32, 32)
